# revision 7
# baseline (speedup 1.0000x reference)
"""AttentionBlock (GroupNorm + single-head self-attention + proj + residual)
on 8 Trainium2 NeuronCores.

Sharding: batch (4) x query-token-half (2) -> 8 shards. Each core gets the
full image of its batch element (for GroupNorm stats and K/V over all 4096
tokens) plus its half of the query tokens; K/V/GN are computed redundantly
by the 2 cores sharing a batch element, which is far cheaper than
cross-core collectives at this size.

Math per core (c=256 channels, n=4096 tokens, nq=2048 query tokens):
  GroupNorm is folded into the QKV weights: xn = s_c * x + t_c with
  per-channel s,t computed on-device from group stats, so
  Q = (wq*s) @ x + (wq@t + bq), etc. The score scale 1/sqrt(c) is folded
  into wk/bk on the host.
  Scores are computed k-major: S^T[m,i] = sum_o K[o,m] Q[o,i] so softmax's
  denominator needs a cross-partition sum, done by accumulating exp tiles
  on DVE and one ones-vector matmul; A@V uses lhsT = V^T (computed directly
  as x^T @ wv') so no transposes are needed anywhere.

All matmuls run in float32r (TF32-like: fp32 with 11-bit mantissa, full
fp32 accumulate) which streams at 1 element/cycle like bf16 -- measured
~1.6e-4 relative error vs fp32, ~15x better than bf16.
"""

import numpy as np

B, C, H, W = 4, 256, 64, 64
N = H * W            # 4096 tokens
NQ = N // 2          # 2048 query tokens per core
GROUPS = 8
GSIZE = C // GROUPS  # 32 channels per group
EPS = 1e-5
P = 128              # partitions
CC = C // P          # 2 channel chunks
NCORES = 8

_cache = {}


def round_tf32(x: np.ndarray) -> np.ndarray:
    """Round fp32 to fp32r (11-bit mantissa, round-to-nearest-even)."""
    i = np.ascontiguousarray(x, dtype=np.float32).view(np.uint32)
    r = (i + np.uint32(0x7FF) + ((i >> np.uint32(12)) & np.uint32(1))) & np.uint32(0xFFFFF000)
    return r.view(np.float32)


def build_nc():
    import concourse.bass as bass
    import concourse.mybir as mybir
    import concourse.tile as tile
    from concourse import bacc

    F32 = mybir.dt.float32
    F32R = mybir.dt.float32r
    AF = mybir.ActivationFunctionType
    OP = mybir.AluOpType

    nc = bacc.Bacc(None, target_bir_lowering=False)

    # ---------- I/O ----------
    x_d = nc.dram_tensor("x_r", [C, N], F32R, kind="ExternalInput")
    xq_d = nc.dram_tensor("xq_r", [C, NQ], F32R, kind="ExternalInput")
    w_d = {}
    b_d = {}
    for nm in ("wq", "wk", "wv", "wp"):
        w_d[nm] = nc.dram_tensor(nm + "_t", [C, C], F32, kind="ExternalInput")
    for nm in ("bq", "bk", "bv", "bp"):
        b_d[nm] = nc.dram_tensor(nm + "_v", [C, 1], F32, kind="ExternalInput")
    gam_d = nc.dram_tensor("gamma_v", [C, 1], F32, kind="ExternalInput")
    bet_d = nc.dram_tensor("beta_v", [C, 1], F32, kind="ExternalInput")
    y_d = nc.dram_tensor("y", [C, NQ], F32, kind="ExternalOutput")

    # group-indicator constants (as fp32r via bitcast; 1/32 and 1 are exact)
    ind1_np = np.zeros((P, 4), dtype=np.float32)
    for c in range(P):
        ind1_np[c, c // GSIZE] = 1.0 / GSIZE
    ind2_np = np.zeros((4, P), dtype=np.float32)
    for c in range(P):
        ind2_np[c // GSIZE, c] = 1.0
    ind1_d = nc.inline_tensor(ind1_np, name="ind1").bitcast(F32R)
    ind2_d = nc.inline_tensor(ind2_np, name="ind2").bitcast(F32R)
    ones_d = nc.inline_tensor(np.ones((P, 1), np.float32), name="ones1").bitcast(F32R)

    NI = N // P        # 32 key-token chunks
    NQ4 = NQ // 512    # 4 query column chunks

    with tile.TileContext(nc) as tc:
        with tc.tile_pool(name="persist", bufs=1) as pp, \
             tc.tile_pool(name="small", bufs=2) as sp, \
             tc.tile_pool(name="work", bufs=2) as wp_pool, \
             tc.tile_pool(name="etp", bufs=4) as etp, \
             tc.tile_pool(name="ps_big", bufs=3, space="PSUM") as psb, \
             tc.tile_pool(name="ps_av", bufs=2, space="PSUM") as psav, \
             tc.tile_pool(name="ps_small", bufs=2, space="PSUM") as pss:

            # ---------- load ----------
            xs = []
            xqs = []
            for cc in range(CC):
                t = pp.tile([P, N], F32R, name=f"xs{cc}")
                nc.sync.dma_start(out=t, in_=x_d[cc * P:(cc + 1) * P, :])
                xs.append(t)
                tq = pp.tile([P, NQ], F32R, name=f"xqs{cc}")
                nc.sync.dma_start(out=tq, in_=xq_d[cc * P:(cc + 1) * P, :])
                xqs.append(tq)
            wraw = {}
            for nm in ("wq", "wk", "wv", "wp"):
                for cc in range(CC):
                    t = pp.tile([P, C], F32, name=f"{nm}raw{cc}")
                    nc.sync.dma_start(out=t, in_=w_d[nm][cc * P:(cc + 1) * P, :])
                    wraw[(nm, cc)] = t
            vecs = {}
            for nm, d in (("bq", b_d["bq"]), ("bk", b_d["bk"]), ("bv", b_d["bv"]),
                          ("bp", b_d["bp"]), ("gam", gam_d), ("bet", bet_d)):
                for cc in range(CC):
                    t = pp.tile([P, 1], F32, name=f"{nm}v{cc}")
                    nc.sync.dma_start(out=t, in_=d[cc * P:(cc + 1) * P, :])
                    vecs[(nm, cc)] = t
            ind1_s = pp.tile([P, 4], F32R, name="ind1s")
            nc.sync.dma_start(out=ind1_s, in_=ind1_d[:, :])
            ind2_s = pp.tile([4, P], F32R, name="ind2s")
            nc.sync.dma_start(out=ind2_s, in_=ind2_d[:, :])
            ones_r = pp.tile([P, 1], F32R, name="ones_r")
            nc.sync.dma_start(out=ones_r, in_=ones_d[:, :])
            eps4 = pp.tile([4, 1], F32, name="eps4")
            nc.vector.memset(eps4, EPS)

            # ---------- GroupNorm stats -> per-channel scale/shift ----------
            s_vecs = []   # [128,1] f32 per cc: s_c = rstd_g * gamma_c
            t_vecs = []   # [128,1] f32 per cc: t_c = beta_c - mean_g * s_c
            for cc in range(CC):
                xf = xs[cc].bitcast(F32)
                stats = sp.tile([P, 8, 6], F32, name="bnstats")
                for sg in range(8):
                    nc.vector.bn_stats(out=stats[:, sg, :], in_=xf[:, sg * 512:(sg + 1) * 512])
                mv = sp.tile([P, 2], F32, name="bnmv")
                nc.vector.bn_aggr(out=mv, in_=stats)
                # st2 = (mean, E[x^2]) per channel, as fp32r
                m2 = sp.tile([P, 1], F32, name="gnm2")
                nc.vector.tensor_mul(out=m2, in0=mv[:, 0:1], in1=mv[:, 0:1])
                st2 = sp.tile([P, 2], F32R, name="gnst2")
                nc.vector.tensor_copy(out=st2[:, 0:1], in_=mv[:, 0:1])
                nc.vector.tensor_tensor(out=st2[:, 1:2], in0=mv[:, 1:2], in1=m2, op=OP.add)
                # group means of (mean, E[x^2]) via indicator matmul
                pg = pss.tile([4, 2], F32, name="psg", tag="pssm")
                nc.tensor.matmul(pg, ind1_s, st2, start=True, stop=True)
                pgs = sp.tile([4, 2], F32, name="gnpgs")
                nc.vector.tensor_copy(out=pgs, in_=pg)
                gm2 = sp.tile([4, 1], F32, name="gngm2")
                nc.vector.tensor_mul(out=gm2, in0=pgs[:, 0:1], in1=pgs[:, 0:1])
                gvar = sp.tile([4, 1], F32, name="gnvar")
                nc.vector.tensor_tensor(out=gvar, in0=pgs[:, 1:2], in1=gm2, op=OP.subtract)
                gstd = sp.tile([4, 1], F32, name="gnstd")
                nc.scalar.activation(out=gstd, in_=gvar, func=AF.Sqrt, bias=eps4, scale=1.0)
                grstd = sp.tile([4, 1], F32, name="gnrstd")
                nc.vector.reciprocal(out=grstd, in_=gstd)
                gvals = sp.tile([4, 2], F32R, name="gnvals")
                nc.vector.tensor_copy(out=gvals[:, 0:1], in_=pgs[:, 0:1])
                nc.vector.tensor_copy(out=gvals[:, 1:2], in_=grstd)
                # broadcast group (mean, rstd) back to channels
                pb = pss.tile([P, 2], F32, name="psb2", tag="pssm")
                nc.tensor.matmul(pb, ind2_s, gvals, start=True, stop=True)
                s_v = sp.tile([P, 1], F32, name="gns")
                nc.vector.tensor_mul(out=s_v, in0=pb[:, 1:2], in1=vecs[("gam", cc)])
                ms = sp.tile([P, 1], F32, name="gnms")
                nc.vector.tensor_mul(out=ms, in0=pb[:, 0:1], in1=s_v)
                t_v = sp.tile([P, 1], F32, name="gnt")
                nc.vector.tensor_tensor(out=t_v, in0=vecs[("bet", cc)], in1=ms, op=OP.subtract)
                s_vecs.append(s_v)
                t_vecs.append(t_v)

            # ---------- fold GN into weights; effective biases ----------
            wr = {}
            for nm in ("wq", "wk", "wv"):
                for cc in range(CC):
                    t = pp.tile([P, C], F32R, name=f"{nm}r{cc}")
                    nc.vector.tensor_scalar_mul(out=t, in0=wraw[(nm, cc)], scalar1=s_vecs[cc])
                    wr[(nm, cc)] = t
            for cc in range(CC):
                t = pp.tile([P, C], F32R, name=f"wpr{cc}")
                nc.vector.tensor_copy(out=t, in_=wraw[("wp", cc)])
                wr[("wp", cc)] = t

            beff = {}
            for nm in ("wq", "wk", "wv"):
                bnm = "b" + nm[1]
                for oc in range(CC):
                    pbx = pss.tile([P, 1], F32, name="psbias", tag="pssm")
                    for cc in range(CC):
                        # raw (unfolded) weights: bias is w @ t, not (w*s) @ t.
                        # fp32 matmul is fine here (N=1).
                        nc.tensor.matmul(pbx, wraw[(nm, cc)][:, oc * P:(oc + 1) * P],
                                         t_vecs[cc], start=(cc == 0), stop=(cc == CC - 1))
                    t = pp.tile([P, 1], F32, name=f"beff_{nm}{oc}")
                    nc.scalar.activation(out=t, in_=pbx, func=AF.Identity,
                                         bias=vecs[(bnm, oc)], scale=1.0)
                    beff[(nm, oc)] = t

            # ---------- projections ----------
            Qs = [pp.tile([P, NQ], F32R, name=f"Q{oc}") for oc in range(CC)]
            Ks = [pp.tile([P, N], F32R, name=f"K{oc}") for oc in range(CC)]
            for oc in range(CC):
                for i in range(NQ4):
                    pq = psb.tile([P, 512], F32, name="psq", tag="pst")
                    for cc in range(CC):
                        nc.tensor.matmul(pq, wr[("wq", cc)][:, oc * P:(oc + 1) * P],
                                         xqs[cc][:, i * 512:(i + 1) * 512],
                                         start=(cc == 0), stop=(cc == CC - 1))
                    nc.scalar.activation(out=Qs[oc][:, i * 512:(i + 1) * 512], in_=pq,
                                         func=AF.Identity, bias=beff[("wq", oc)], scale=1.0)
                for i in range(N // 512):
                    pk = psb.tile([P, 512], F32, name="psk", tag="pst")
                    for cc in range(CC):
                        nc.tensor.matmul(pk, wr[("wk", cc)][:, oc * P:(oc + 1) * P],
                                         xs[cc][:, i * 512:(i + 1) * 512],
                                         start=(cc == 0), stop=(cc == CC - 1))
                    nc.scalar.activation(out=Ks[oc][:, i * 512:(i + 1) * 512], in_=pk,
                                         func=AF.Identity, bias=beff[("wk", oc)], scale=1.0)
            VTs = pp.tile([P, NI * C], F32R, name="VTs")  # [128 tok, 32*256]
            for it in range(NI):
                pv = psb.tile([P, C], F32, name="psv", tag="pst")
                for cc in range(CC):
                    nc.tensor.matmul(pv, xs[cc][:, it * P:(it + 1) * P], wr[("wv", cc)],
                                     start=(cc == 0), stop=(cc == CC - 1))
                nc.vector.tensor_copy(out=VTs[:, it * C:(it + 1) * C], in_=pv)

            # ---------- attention ----------
            for qi in range(NQ4):
                pav = [psav.tile([P, 512], F32, name=f"pav{cc}", tag="pav") for cc in range(CC)]
                acc_d = wp_pool.tile([P, 512], F32, name="acc_d", tag="acc_d")
                acc_r = wp_pool.tile([P, 512], F32R, name="acc_r", tag="acc_r")
                for m in range(NI):
                    pst = psb.tile([P, 512], F32, name="pst", tag="pst")
                    for oc in range(CC):
                        nc.tensor.matmul(pst, Ks[oc][:, m * P:(m + 1) * P],
                                         Qs[oc][:, qi * 512:(qi + 1) * 512],
                                         start=(oc == 0), stop=(oc == CC - 1))
                    et = etp.tile([P, 512], F32R, name="et", tag="et")
                    nc.scalar.activation(out=et, in_=pst, func=AF.Exp)
                    for cc in range(CC):
                        nc.tensor.matmul(pav[cc], VTs[:, m * C + cc * P: m * C + (cc + 1) * P],
                                         et, start=(m == 0), stop=(m == NI - 1))
                    etf = et.bitcast(F32)
                    if m == 0:
                        nc.vector.tensor_copy(out=acc_d, in_=etf)
                    elif m < NI - 1:
                        nc.vector.tensor_tensor(out=acc_d, in0=acc_d, in1=etf, op=OP.add)
                    else:
                        nc.vector.tensor_tensor(out=acc_r, in0=acc_d, in1=etf, op=OP.add)
                pd = pss.tile([1, 512], F32, name="psd", tag="pssm")
                nc.tensor.matmul(pd, ones_r, acc_r, start=True, stop=True)
                rq = wp_pool.tile([1, 512], F32, name="rq", tag="rq")
                nc.vector.reciprocal(out=rq, in_=pd)
                rb = wp_pool.tile([P, 512], F32, name="rb", tag="rb")
                nc.gpsimd.partition_broadcast(rb, rq)
                obar = []
                for cc in range(CC):
                    ot = wp_pool.tile([P, 512], F32, name="otmp", tag="otmp")
                    nc.vector.tensor_mul(out=ot, in0=pav[cc], in1=rb)
                    ob = wp_pool.tile([P, 512], F32R, name="obar", tag="obar")
                    nc.vector.tensor_scalar_add(out=ob, in0=ot, scalar1=beff[("wv", cc)])
                    obar.append(ob)
                for oc in range(CC):
                    py = psb.tile([P, 512], F32, name="psy", tag="pst")
                    for cc in range(CC):
                        nc.tensor.matmul(py, wr[("wp", cc)][:, oc * P:(oc + 1) * P],
                                         obar[cc], start=(cc == 0), stop=(cc == CC - 1))
                    y1 = wp_pool.tile([P, 512], F32, name="y1", tag="y1")
                    nc.scalar.activation(out=y1, in_=py, func=AF.Identity,
                                         bias=vecs[("bp", oc)], scale=1.0)
                    y2 = wp_pool.tile([P, 512], F32, name="y2", tag="y2")
                    nc.vector.tensor_tensor(out=y2, in0=y1,
                                            in1=xqs[oc].bitcast(F32)[:, qi * 512:(qi + 1) * 512],
                                            op=OP.add)
                    nc.sync.dma_start(out=y_d[oc * P:(oc + 1) * P, qi * 512:(qi + 1) * 512],
                                      in_=y2)

    nc.finalize()
    return nc


def _get_nc():
    if "nc" not in _cache:
        _cache["nc"] = build_nc()
    return _cache["nc"]


def make_in_maps(x, gamma, beta, wq, bq, wk, bk, wv, bv, wp, bp):
    x = np.ascontiguousarray(np.asarray(x, dtype=np.float32))
    f32 = lambda a: np.ascontiguousarray(np.asarray(a, dtype=np.float32))
    scale = 1.0 / np.sqrt(np.float32(C))
    shared = {
        "wq_t": f32(np.asarray(wq, np.float32).T),
        "wk_t": f32(np.asarray(wk, np.float32).T * scale),
        "wv_t": f32(np.asarray(wv, np.float32).T),
        "wp_t": f32(np.asarray(wp, np.float32).T),
        "bq_v": f32(bq).reshape(C, 1),
        "bk_v": f32(np.asarray(bk, np.float32) * scale).reshape(C, 1),
        "bv_v": f32(bv).reshape(C, 1),
        "bp_v": f32(bp).reshape(C, 1),
        "gamma_v": f32(gamma).reshape(C, 1),
        "beta_v": f32(beta).reshape(C, 1),
    }
    in_maps = []
    for core in range(NCORES):
        bi, half = core // 2, core % 2
        x_r = round_tf32(x[bi].reshape(C, N))
        xq_r = np.ascontiguousarray(x_r[:, half * NQ:(half + 1) * NQ])
        m = dict(shared)
        m["x_r"] = x_r
        m["xq_r"] = xq_r
        in_maps.append(m)
    return in_maps


def run(inputs: dict, trace: bool = False):
    from concourse.bass_utils import run_bass_kernel_spmd
    nc = _get_nc()
    in_maps = make_in_maps(**inputs)
    res = run_bass_kernel_spmd(nc, in_maps, core_ids=list(range(NCORES)), trace=trace)
    y = np.empty((B, C, N), dtype=np.float32)
    for core in range(NCORES):
        bi, half = core // 2, core % 2
        y[bi][:, half * NQ:(half + 1) * NQ] = res.results[core]["y"]
    return y.reshape(B, C, H, W), res


def kernel(**inputs) -> np.ndarray:
    out, _ = run(inputs, trace=False)
    return out
